# revision 11
# baseline (speedup 1.0000x reference)
"""Two-layer GAT + linear head + log_softmax on 8 Trainium2 NeuronCores.

Design (v2):
  - Nodes sharded 12500/core; per-core tiles of 128 nodes chosen by
    lexsorting per-window edge-count vectors (padding ~1.5x).
  - Gather tables hold bf16 rows of 256 (L0) / 128 (L1) ROTATED features:
    table = h @ (Q D) with Q orthogonal (Householder) whose last column is
    a_dst/||a_dst|| and D scaling that column by ||a_dst||; the per-edge
    attention score s_dst rides in the last column, and the weighted sum is
    un-rotated once per tile by a PE matmul with M^-1 (exact, linear).
  - Per (tile-group, window) ONE dma_gather instruction fetches all edge
    rows (int16 idx => 4 overlapping 32768-row windows of the table).
  - L0 table rows in id order (dense phase scatters via indirect DMA) so
    window membership is independent of the tile sort; L1 table in sorted
    order (contiguous writes), AllGathered in 4 chunks overlapped with the
    edge0 phase.
  - Weighted sums use dual bf16 accumulators on DVE combined in f32.

Self-contained: hardcodes N=100000, E=3200000, 8 cores.
"""

import numpy as np
import ml_dtypes

BF16 = ml_dtypes.bfloat16

NC = 8
P = 128
N = 100000
E = 3200000
SH = N // NC                # 12500
T = (SH + P - 1) // P       # 98
POS = T * P                 # 12544
ALPHA = 0.2
NEG = -1.0e30
W = 32768
NW = 4

# L0 table: id-order rows via scatter; [12500 nodes][pad][trash] per shard
SHP0 = SH + 2               # 12502
NT0 = NC * SHP0             # 100016
BASES0 = (0, 22416, 44832, 67248)
PADROW0 = (12500, 25002, 50006, 75010)
TRASH0 = SH + 1

# L1 table: sorted-order rows, 4 chunks of (3125 rows + pad) per shard
CH = 4
CROWS = SH // CH            # 3125
SHP1 = CH * (CROWS + 1)     # 12504
NT1 = NC * SHP1             # 100032
CPG1 = NC * (CROWS + 1)     # 25008
BASES1 = (0, 22422, 44844, 67264)
PADROW1 = (3125, 25007, 50015, 75023)

JCAP = 96                   # max slots per tile-group (SBUF bound)
QCAP = 42                   # max slots per (group, window) (Q7 scratch bound)
AGT_TILES = (24, 48, 73, 97)   # dense1 tiles after which AG1 chunks fire


def _check(cond, msg):
    if not cond:
        raise AssertionError(msg)


def _householder_M(a_dst):
    d = a_dst.shape[0]
    a = a_dst.reshape(-1).astype(np.float64)
    nrm = np.linalg.norm(a)
    u = a / nrm
    e = np.zeros(d)
    e[-1] = 1.0
    v = u - e
    vn = np.linalg.norm(v)
    if vn < 1e-12:
        Q = np.eye(d)
    else:
        v = v / vn
        Q = np.eye(d) - 2.0 * np.outer(v, v)
    M = Q.copy()
    M[:, -1] *= nrm
    Minv = np.linalg.inv(M)
    return M.astype(np.float32), Minv.astype(np.float32)


def _preprocess(edge):
    src = np.asarray(edge[0], np.int64)
    dst = np.asarray(edge[1], np.int64)
    Ee = src.shape[0]
    deg = np.bincount(src, minlength=N)
    _check(deg.min() >= 1, "empty rows unsupported")

    core_v = np.arange(N) // SH
    idp = np.arange(N) % SH
    trow0 = core_v * SHP0 + idp
    drow0 = trow0[dst]
    b0 = np.asarray(BASES0)
    lo0 = np.searchsorted(b0 + W, drow0, side="right").astype(np.int8)
    hi0 = (np.searchsorted(b0, drow0, side="right") - 1).astype(np.int8)
    _check((lo0 <= hi0).all(), "window coverage hole")

    # ---- per-node greedy balance on L0 windows -> count vectors ----
    o = np.argsort(src, kind="stable")
    l_s, h_s = lo0[o], hi0[o]
    starts = np.searchsorted(src[o], np.arange(N + 1))
    cntv = np.zeros((N, NW), np.int16)
    for v in range(N):
        a, b = starts[v], starts[v + 1]
        l, h = l_s[a:b], h_s[a:b]
        c = np.bincount(l[l == h], minlength=NW).astype(np.int32)
        for i in np.where(l != h)[0]:
            q = l[i] + int(np.argmin(c[l[i]:h[i] + 1]))
            c[q] += 1
        cntv[v] = c

    # ---- per-core lexsort of count vectors -> tile composition ----
    pos = np.empty(N, np.int64)
    perm = np.empty((NC, SH), np.int64)
    for c in range(NC):
        nodes = np.arange(c * SH, (c + 1) * SH)
        vv = cntv[nodes]
        oo = np.lexsort((vv[:, 3], vv[:, 2], vv[:, 1], vv[:, 0]))
        pos[nodes[oo]] = np.arange(SH)
        perm[c] = nodes[oo]

    # L1 table rows (sorted order, chunked)
    kch = pos // CROWS
    trow1 = kch * CPG1 + core_v * (CROWS + 1) + (pos % CROWS)
    drow1 = trow1[dst]
    b1 = np.asarray(BASES1)
    lo1 = np.searchsorted(b1 + W, drow1, side="right").astype(np.int8)
    hi1 = (np.searchsorted(b1, drow1, side="right") - 1).astype(np.int8)
    _check((lo1 <= hi1).all(), "window1 coverage hole")

    gpos = core_v[src] * SH + pos[src]
    c_e = (gpos // SH).astype(np.int32)
    p_loc = gpos % SH
    t_e = (p_loc // P).astype(np.int32)
    prow = (p_loc % P).astype(np.int32)
    tileid = c_e * T + t_e

    o2 = np.argsort(tileid * P + prow, kind="stable")
    bounds = np.searchsorted(tileid[o2], np.arange(NC * T + 1))

    def assign(lo_w, hi_w):
        """Per-tile joint greedy window assignment; returns win, KHQ."""
        win = np.empty(Ee, np.int8)
        KHQ = np.zeros((T, NW), np.int32)
        l2, h2, p2 = lo_w[o2], hi_w[o2], prow[o2]
        for tl in range(NC * T):
            a, b = bounds[tl], bounds[tl + 1]
            if a == b:
                continue
            t = tl % T
            ll, hh, pp = l2[a:b], h2[a:b], p2[a:b]
            cnt = np.zeros((P, NW), np.int32)
            forced = ll == hh
            np.add.at(cnt, (pp[forced], ll[forced]), 1)
            win[o2[a:b][forced]] = ll[forced]
            wmax = cnt.max(axis=0)
            for i in np.where(~forced)[0]:
                p_, lq, hq = pp[i], ll[i], hh[i]
                bq, bk = lq, None
                for q in range(lq, hq + 1):
                    nv = cnt[p_, q] + 1
                    kk = (max(0, nv - wmax[q]), nv)
                    if bk is None or kk < bk:
                        bk, bq = kk, q
                win[o2[a + i]] = bq
                cnt[p_, bq] += 1
                if cnt[p_, bq] > wmax[bq]:
                    wmax[bq] = cnt[p_, bq]
            KHQ[t] = np.maximum(KHQ[t], cnt.max(axis=0))
        return win, np.maximum(KHQ, 0)

    win0, KHQ0 = assign(lo0, hi0)
    win1, KHQ1 = assign(lo1, hi1)

    def ranks(win):
        k2 = (tileid.astype(np.int64) * P + prow) * NW + win
        o3 = np.argsort(k2, kind="stable")
        ks = k2[o3]
        newgrp = np.r_[True, ks[1:] != ks[:-1]]
        startpos = np.maximum.accumulate(np.where(newgrp, np.arange(Ee), 0))
        r = np.arange(Ee) - startpos
        rank = np.empty(Ee, np.int32)
        rank[o3] = r.astype(np.int32)
        return rank

    rank0 = ranks(win0)
    rank1 = ranks(win1)

    # ---- fuse tiles into groups ----
    groups = []
    cur, js = [], 0
    qs0 = np.zeros(NW, np.int64)
    qs1 = np.zeros(NW, np.int64)
    Jt0 = KHQ0.sum(axis=1)
    for t in range(T):
        jt = int(Jt0[t])
        if cur and (js + jt > JCAP or (qs0 + KHQ0[t]).max() > QCAP
                    or (qs1 + KHQ1[t]).max() > QCAP):
            groups.append(cur)
            cur, js = [], 0
            qs0 = np.zeros(NW, np.int64)
            qs1 = np.zeros(NW, np.int64)
        cur.append(t)
        js += jt
        qs0 = qs0 + KHQ0[t]
        qs1 = qs1 + KHQ1[t]
    if cur:
        groups.append(cur)

    def layout(KHQ):
        """Per-group per-window widths + per-(t,q) slot offsets + call cols."""
        GQW = []          # [g][q] width in slots
        CQOFF = {}        # (t,q) -> slot offset within call (tile-major)
        for g, tl in enumerate(groups):
            w = np.zeros(NW, np.int64)
            for q in range(NW):
                off = 0
                for t in tl:
                    CQOFF[(t, q)] = off
                    off += int(KHQ[t][q])
                w[q] = off
            GQW.append(w)
        # free-col offset of each call's idx block in the eidx tensor,
        # and slot offset of each call in the global slot layout
        callfree = {}
        callslot = {}
        fo = 0
        so = 0
        for g in range(len(groups)):
            for q in range(NW):
                callfree[(g, q)] = fo
                callslot[(g, q)] = so
                fo += int(GQW[g][q]) * 8
                so += int(GQW[g][q])
        return GQW, CQOFF, callfree, callslot, fo, so

    GQW0, CQOFF0, CF0, CS0, F0, S0 = layout(KHQ0)
    GQW1, CQOFF1, CF1, CS1, F1, S1 = layout(KHQ1)

    gidx_of_tile = np.zeros(T, np.int32)
    for g, tl in enumerate(groups):
        for t in tl:
            gidx_of_tile[t] = g

    def build_eidx(win, rank, GQW, drow, bases, padrows, CQOFF, CS, TOT):
        bases = np.asarray(bases)
        padrows = np.asarray(padrows)
        # per-call pad value for every global slot col
        padval = np.empty(TOT, np.int16)
        for g in range(len(groups)):
            for q in range(NW):
                s0 = CS[(g, q)]
                wdt = int(GQW[g][q])
                padval[s0:s0 + wdt] = padrows[q] - bases[q]
        eidx = [None] * NC
        colcq = np.zeros((T, NW), np.int64)
        for t in range(T):
            g = gidx_of_tile[t]
            for q in range(NW):
                colcq[t, q] = CS[(g, q)] + CQOFF[(t, q)]
        cols_e = colcq[t_e, win] + rank
        v64 = drow - bases[win]
        _check(v64.min() >= 0 and v64.max() <= 32767, "idx out of int16 range")
        vals_e = v64.astype(np.int16)
        for c in range(NC):
            m = c_e == c
            val = np.repeat(padval[:, None], P, axis=1)
            val[cols_e[m], prow[m]] = vals_e[m]
            # wrapped layout: flat i = col*128+p -> [p%16, col*8 + p//16]
            sb = val.reshape(TOT, 8, 16).transpose(2, 0, 1).reshape(16, TOT * 8)
            eidx[c] = np.ascontiguousarray(np.tile(sb, (8, 1)).astype(np.int16))
        return eidx

    eidx0 = build_eidx(win0, rank0, GQW0, drow0, BASES0, PADROW0,
                       CQOFF0, CS0, S0)
    eidx1 = build_eidx(win1, rank1, GQW1, drow1, BASES1, PADROW1,
                       CQOFF1, CS1, S1)

    # scatter rows for dense0 (per core): node at sorted pos -> id-order row
    scidx = np.full((NC, P, T), TRASH0, np.int32)
    for c in range(NC):
        idps = perm[c] % SH   # id offset of node at sorted position
        sc = idps.astype(np.int32)
        scidx[c, :, :] = np.pad(sc, (0, POS - SH),
                                constant_values=TRASH0).reshape(T, P).T

    # dense1 write pieces per tile: (sbuf_row, sh1_row, n)
    pieces = []
    for t in range(T):
        lo = t * P
        hi = min(lo + P, SH)
        pcs = []
        r = lo
        while r < hi:
            k = r // CROWS
            end = min(hi, (k + 1) * CROWS)
            pcs.append((r - lo, k * (CROWS + 1) + (r % CROWS), end - r))
            r = end
        pieces.append(pcs)

    # per-tile slot maps for the kernel: for tile t, ordered slot columns
    # (within its group's gbuf) and J(t)
    tslots = []
    for t in range(T):
        g = gidx_of_tile[t]
        base_g0 = CS0[(g, 0)]   # slot offset of group g's first call
        cols0, cols1 = [], []
        for q in range(NW):
            o0 = CS0[(g, q)] - CS0[(g, 0)] + CQOFF0[(t, q)]
            cols0.append((o0, int(KHQ0[t][q])))
            o1 = CS1[(g, q)] - CS1[(g, 0)] + CQOFF1[(t, q)]
            cols1.append((o1, int(KHQ1[t][q])))
        tslots.append((cols0, cols1))

    meta = dict(
        groups=groups, gidx=gidx_of_tile,
        KHQ0=KHQ0, KHQ1=KHQ1, GQW0=GQW0, GQW1=GQW1,
        CF0=CF0, CF1=CF1, CS0=CS0, CS1=CS1, F0=F0, F1=F1,
        tslots=tslots, pieces=pieces,
    )
    return dict(eidx0=eidx0, eidx1=eidx1, scidx=scidx, perm=perm, meta=meta)


def _build(meta, variant="full"):
    import concourse.bacc as bacc
    import concourse.bass as bass
    import concourse.mybir as mybir
    from concourse.tile import TileContext
    from concourse.masks import make_identity

    dt = mybir.dt
    AF = mybir.ActivationFunctionType
    ALU = mybir.AluOpType

    groups = meta["groups"]
    gidx = meta["gidx"]
    KHQ0, KHQ1 = meta["KHQ0"], meta["KHQ1"]
    GQW0, GQW1 = meta["GQW0"], meta["GQW1"]
    CF0, CF1 = meta["CF0"], meta["CF1"]
    CS0, CS1 = meta["CS0"], meta["CS1"]
    F0, F1 = meta["F0"], meta["F1"]
    pieces, tslots = meta["pieces"], meta["tslots"]

    nc = bacc.Bacc(num_swdge_queues=4)

    xT = nc.declare_dram_parameter("xT", [256, POS], dt.float32, isOutput=False)
    eidx0_d = nc.declare_dram_parameter("eidx0", [128, F0], dt.int16, isOutput=False)
    eidx1_d = nc.declare_dram_parameter("eidx1", [128, F1], dt.int16, isOutput=False)
    scidx_d = nc.declare_dram_parameter("scidx", [128, T], dt.int32, isOutput=False)
    w0e = nc.declare_dram_parameter("w0e", [256, 257], dt.float32, isOutput=False)
    w1e = nc.declare_dram_parameter("w1e", [256, 129], dt.float32, isOutput=False)
    m0i_d = nc.declare_dram_parameter("m0i", [256, 256], dt.float32, isOutput=False)
    m1i_d = nc.declare_dram_parameter("m1i", [128, 128], dt.float32, isOutput=False)
    lw = nc.declare_dram_parameter("lw", [128, 40], dt.float32, isOutput=False)
    lb = nc.declare_dram_parameter("lb", [128, 40], dt.float32, isOutput=False)
    pad0_d = nc.declare_dram_parameter("pad0", [1, 256], dt.bfloat16, isOutput=False)
    pad1_d = nc.declare_dram_parameter("pad1", [1, 128], dt.bfloat16, isOutput=False)
    logits = nc.declare_dram_parameter("logits", [POS, 40], dt.float32, isOutput=True)

    sh0 = nc.dram_tensor("sh0", [SHP0, 256], dt.bfloat16)
    t0 = nc.dram_tensor("t0", [NT0, 256], dt.bfloat16, addr_space="Shared")
    sh1 = nc.dram_tensor("sh1", [SHP1, 128], dt.bfloat16)
    t1 = nc.dram_tensor("t1", [NT1, 128], dt.bfloat16, addr_space="Shared")

    rg = [list(range(NC))]
    NG = len(groups)

    with TileContext(nc) as tc:
        with (
            tc.tile_pool(name="const", bufs=1) as constp,
            tc.tile_pool(name="gpool", bufs=2) as gpool,
            tc.tile_pool(name="ipool", bufs=2) as ipool,
            tc.tile_pool(name="spool", bufs=3) as spool,
            tc.tile_pool(name="hpool", bufs=3) as hpool,
            tc.tile_pool(name="xpool", bufs=4) as xpool,
            tc.tile_pool(name="psA", bufs=2, space="PSUM") as psA,
            tc.tile_pool(name="psT", bufs=2, space="PSUM") as psT,
        ):
            # ---- resident constants ----
            w0a = constp.tile([128, 257], dt.float32, tag="w0a")
            w0b = constp.tile([128, 257], dt.float32, tag="w0b")
            w1a = constp.tile([128, 129], dt.float32, tag="w1a")
            w1b = constp.tile([128, 129], dt.float32, tag="w1b")
            m0ia = constp.tile([128, 256], dt.float32, tag="m0ia")
            m0ib = constp.tile([128, 256], dt.float32, tag="m0ib")
            m1i = constp.tile([128, 128], dt.float32, tag="m1i")
            lwt = constp.tile([128, 40], dt.float32, tag="lwt")
            lbt = constp.tile([128, 40], dt.float32, tag="lbt")
            ident = constp.tile([128, 128], dt.float32, tag="ident")
            ssrc0 = constp.tile([128, T], dt.float32, tag="ssrc0")
            ssrc1 = constp.tile([128, T], dt.float32, tag="ssrc1")
            scid = constp.tile([128, T], dt.int32, tag="scid")
            padt0 = constp.tile([1, 256], dt.bfloat16, tag="padt0")
            padt1 = constp.tile([1, 128], dt.bfloat16, tag="padt1")

            nc.sync.dma_start(out=w0a[:], in_=w0e[0:128, :])
            nc.sync.dma_start(out=w0b[:], in_=w0e[128:256, :])
            nc.sync.dma_start(out=w1a[:], in_=w1e[0:128, :])
            nc.sync.dma_start(out=w1b[:], in_=w1e[128:256, :])
            nc.sync.dma_start(out=m0ia[:], in_=m0i_d[0:128, :])
            nc.sync.dma_start(out=m0ib[:], in_=m0i_d[128:256, :])
            nc.sync.dma_start(out=m1i[:], in_=m1i_d[:, :])
            nc.sync.dma_start(out=lwt[:], in_=lw[:, :])
            nc.sync.dma_start(out=lbt[:], in_=lb[:, :])
            nc.sync.dma_start(out=scid[:], in_=scidx_d[:, :])
            nc.sync.dma_start(out=padt0[:], in_=pad0_d[:, :])
            nc.sync.dma_start(out=padt1[:], in_=pad1_d[:, :])
            make_identity(nc, ident[:])
            # pad rows: L0 one per shard (local row 12500); L1 one per chunk
            nc.sync.dma_start(out=sh0[SH:SH + 1, :], in_=padt0[:])
            for k in range(CH):
                r = k * (CROWS + 1) + CROWS
                nc.sync.dma_start(out=sh1[r:r + 1, :], in_=padt1[:])

            # ---- dense0: hE = x @ [W0 M0 | W0 a0src], scatter to sh0 ----
            for t in range(T):
                xa = xpool.tile([128, 128], dt.float32, tag="xa")
                xb = xpool.tile([128, 128], dt.float32, tag="xb")
                cols = slice(t * P, (t + 1) * P)
                nc.sync.dma_start(out=xa[:], in_=xT[0:128, cols])
                nc.sync.dma_start(out=xb[:], in_=xT[128:256, cols])
                ps = psA.tile([128, 257], dt.float32, tag="apsum")
                nc.tensor.matmul(ps[:], lhsT=xa[:], rhs=w0a[:], start=True, stop=False)
                nc.tensor.matmul(ps[:], lhsT=xb[:], rhs=w0b[:], start=False, stop=True)
                hb = hpool.tile([128, 256], dt.bfloat16, tag="hb")
                nc.scalar.copy(out=hb[:], in_=ps[:, 0:256])
                nc.vector.tensor_copy(out=ssrc0[:, t:t + 1], in_=ps[:, 256:257])
                nc.gpsimd.indirect_dma_start(
                    out=sh0[:, :],
                    out_offset=bass.IndirectOffsetOnAxis(
                        ap=scid[:, t:t + 1], axis=0),
                    in_=hb[:],
                    in_offset=None,
                )

            nc.gpsimd.collective_compute(
                "AllGather", mybir.AluOpType.bypass,
                ins=[sh0[:]], outs=[t0[:]], replica_groups=rg,
            )

            # ---- helpers ----
            qrot = [0]

            def prefetch(g, layer):
                """Load idx slice + issue the window gathers for group g,
                split into <=8-slot-column calls rotated over 4 SWDGE queues."""
                tl = groups[g]
                if layer == 0:
                    GQW, CF, CS, table, bases, dcols = (
                        GQW0, CF0, CS0, t0, BASES0, 256)
                    eidx_d_, JG = eidx0_d, int(sum(KHQ0[t].sum() for t in tl))
                else:
                    GQW, CF, CS, table, bases, dcols = (
                        GQW1, CF1, CS1, t1, BASES1, 128)
                    eidx_d_, JG = eidx1_d, int(sum(KHQ1[t].sum() for t in tl))
                f_lo = CF[(g, 0)]
                f_hi = CF[(g, NW - 1)] + int(GQW[g][NW - 1]) * 8
                it = ipool.tile([128, f_hi - f_lo], dt.int16, tag=f"idx{layer}")
                nc.sync.dma_start(out=it[:], in_=eidx_d_[:, f_lo:f_hi])
                gb = gpool.tile([128, JG * dcols], dt.bfloat16, tag=f"g{layer}")
                soff = 0
                for q in range(NW):
                    wdt = int(GQW[g][q])
                    if wdt == 0:
                        continue
                    qf = CF[(g, q)] - f_lo
                    for off in range(0, wdt, 8):
                        wsub = min(8, wdt - off)
                        ni = wsub * P
                        c0 = soff + off
                        nc.gpsimd.dma_gather(
                            gb[:, c0 * dcols:(c0 + wsub) * dcols].rearrange(
                                "p (j e) -> p j e", e=dcols),
                            table[bases[q]:bases[q] + W, :],
                            it[:, qf + off * 8:qf + off * 8 + ni // 16],
                            ni, ni, dcols,
                            queue_num=qrot[0],
                        )
                        qrot[0] = (qrot[0] + 1) % 4
                    soff += wdt
                return gb

            def elu_psum_to(out_tile, ps_c):
                tneg = hpool.tile(list(out_tile.shape), dt.float32, tag="tneg")
                nc.vector.tensor_scalar_min(tneg[:], ps_c[:], 0.0)
                expm = hpool.tile(list(out_tile.shape), dt.float32, tag="expm")
                nc.scalar.activation(out=expm[:], in_=tneg[:], func=AF.Exp, bias=0.0)
                nc.vector.scalar_tensor_tensor(
                    out=out_tile[:], in0=expm[:], scalar=-1.0, in1=ps_c[:],
                    op0=ALU.add, op1=ALU.max,
                )

            def edge_tile(t, gb, layer):
                """Softmax + weighted sum for tile t; returns hn [128, dh] f32."""
                cols = tslots[t][layer]
                dh = 256 if layer == 0 else 128
                ssrc = ssrc0 if layer == 0 else ssrc1
                J = sum(w for _, w in cols)
                g3 = gb[:].rearrange("p (j e) -> p j e", e=dh)
                sd = spool.tile([128, J], dt.float32, tag="sd")
                co = 0
                for o0, wdt in cols:
                    if wdt == 0:
                        continue
                    nc.scalar.copy(
                        out=sd[:, co:co + wdt].rearrange("p (k o) -> p k o", o=1),
                        in_=g3[:, o0:o0 + wdt, dh - 1:dh],
                    )
                    co += wdt
                sc0 = spool.tile([128, J], dt.float32, tag="sc0")
                nc.scalar.activation(
                    out=sc0[:], in_=sd[:], func=AF.Identity,
                    bias=ssrc[:, t:t + 1], scale=1.0,
                )
                sc = spool.tile([128, J], dt.float32, tag="sc")
                nc.vector.scalar_tensor_tensor(
                    out=sc[:], in0=sc0[:], scalar=ALPHA, in1=sc0[:],
                    op0=ALU.mult, op1=ALU.max,
                )
                m = spool.tile([128, 1], dt.float32, tag="m")
                nc.vector.reduce_max(out=m[:], in_=sc[:], axis=mybir.AxisListType.X)
                negm = spool.tile([128, 1], dt.float32, tag="negm")
                nc.vector.tensor_scalar_mul(negm[:], m[:], -1.0)
                wgt = spool.tile([128, J], dt.float32, tag="wgt")
                z = spool.tile([128, 1], dt.float32, tag="z")
                nc.scalar.activation(
                    out=wgt[:], in_=sc[:], func=AF.Exp,
                    bias=negm[:, 0:1], scale=1.0, accum_out=z[:, 0:1],
                )
                rz = spool.tile([128, 1], dt.float32, tag="rz")
                nc.vector.reciprocal(rz[:], z[:])
                # dual bf16 accumulators over the tile's slots
                acc0 = hpool.tile([128, dh], dt.bfloat16, tag="acc0")
                acc1 = hpool.tile([128, dh], dt.bfloat16, tag="acc1")
                slotlist = []
                for o0, wdt in cols:
                    slotlist.extend(range(o0, o0 + wdt))
                for j, so in enumerate(slotlist):
                    tgt = acc0 if j % 2 == 0 else acc1
                    if j < 2:
                        nc.vector.tensor_scalar(
                            out=tgt[:], in0=g3[:, so, 0:dh],
                            scalar1=wgt[:, j:j + 1], scalar2=None, op0=ALU.mult,
                        )
                    else:
                        nc.vector.scalar_tensor_tensor(
                            out=tgt[:], in0=g3[:, so, 0:dh],
                            scalar=wgt[:, j:j + 1], in1=tgt[:],
                            op0=ALU.mult, op1=ALU.add,
                        )
                if len(slotlist) < 2:
                    nc.vector.tensor_scalar(
                        out=acc1[:], in0=g3[:, 0, 0:dh],
                        scalar1=0.0, scalar2=None, op0=ALU.mult,
                    )
                accf = hpool.tile([128, dh], dt.float32, tag="accf")
                nc.vector.tensor_tensor(
                    out=accf[:], in0=acc0[:], in1=acc1[:], op=ALU.add)
                hn = hpool.tile([128, dh], dt.float32, tag="hn")
                nc.scalar.activation(
                    out=hn[:], in_=accf[:], func=AF.Copy,
                    bias=0.0, scale=rz[:, 0:1],
                )
                return hn

            # ---- edge0 (+ fused unrotate/elu + dense1 + chunked AG1) ----
            def run_edge0():
                gbs = {0: prefetch(0, 0)}
                agq = list(AGT_TILES)
                agk = 0
                for g in range(NG):
                    if g + 1 < NG:
                        gbs[g + 1] = prefetch(g + 1, 0)
                    gb = gbs.pop(g)
                    for t in groups[g]:
                        hn = edge_tile(t, gb, 0)
                        # h0T = elu(M0inv^T @ hn^T), two 128-col chunks
                        hnT = []
                        for half in range(2):
                            pt = psT.tile([128, 128], dt.float32, tag="pt")
                            nc.tensor.transpose(
                                pt[:], hn[:, half * 128:(half + 1) * 128], ident[:])
                            tt = xpool.tile([128, 128], dt.float32, tag="hnT")
                            nc.scalar.copy(out=tt[:], in_=pt[:])
                            hnT.append(tt)
                        h0T = []
                        for cc in range(2):
                            csl = slice(cc * 128, (cc + 1) * 128)
                            pc = psT.tile([128, 128], dt.float32, tag="pc")
                            nc.tensor.matmul(pc[:], lhsT=m0ia[:, csl], rhs=hnT[0][:],
                                             start=True, stop=False)
                            nc.tensor.matmul(pc[:], lhsT=m0ib[:, csl], rhs=hnT[1][:],
                                             start=False, stop=True)
                            ht = xpool.tile([128, 128], dt.float32, tag="h0T")
                            elu_psum_to(ht, pc)
                            h0T.append(ht)
                        # dense1 for tile t directly from h0T
                        ps1 = psA.tile([128, 129], dt.float32, tag="apsum")
                        nc.tensor.matmul(ps1[:], lhsT=h0T[0][:], rhs=w1a[:],
                                         start=True, stop=False)
                        nc.tensor.matmul(ps1[:], lhsT=h0T[1][:], rhs=w1b[:],
                                         start=False, stop=True)
                        hb1 = hpool.tile([128, 128], dt.bfloat16, tag="hb1")
                        nc.scalar.copy(out=hb1[:], in_=ps1[:, 0:128])
                        nc.vector.tensor_copy(
                            out=ssrc1[:, t:t + 1], in_=ps1[:, 128:129])
                        for sb_r, sh_r, n in pieces[t]:
                            nc.sync.dma_start(
                                out=sh1[sh_r:sh_r + n, :], in_=hb1[sb_r:sb_r + n, :])
                        while agk < CH and agq and t == agq[0]:
                            k = agk
                            nc.gpsimd.collective_compute(
                                "AllGather", mybir.AluOpType.bypass,
                                ins=[sh1[k * (CROWS + 1):(k + 1) * (CROWS + 1), :]],
                                outs=[t1[k * CPG1:(k + 1) * CPG1, :]],
                                replica_groups=rg,
                            )
                            agq.pop(0)
                            agk += 1

            # ---- edge1 (+ head) ----
            def run_edge1():
                gbs = {0: prefetch(0, 1)}
                for g in range(NG):
                    if g + 1 < NG:
                        gbs[g + 1] = prefetch(g + 1, 1)
                    gb = gbs.pop(g)
                    for t in groups[g]:
                        hn = edge_tile(t, gb, 1)
                        pt = psT.tile([128, 128], dt.float32, tag="pt")
                        nc.tensor.transpose(pt[:], hn[:, 0:128], ident[:])
                        hnT = xpool.tile([128, 128], dt.float32, tag="hnT")
                        nc.scalar.copy(out=hnT[:], in_=pt[:])
                        pc = psT.tile([128, 128], dt.float32, tag="pc")
                        nc.tensor.matmul(pc[:], lhsT=m1i[:], rhs=hnT[:],
                                         start=True, stop=True)
                        h1T = xpool.tile([128, 128], dt.float32, tag="h1T")
                        elu_psum_to(h1T, pc)
                        ps40 = psA.tile([128, 40], dt.float32, tag="apsum")
                        nc.tensor.matmul(ps40[:], lhsT=h1T[:], rhs=lwt[:],
                                         start=True, stop=True)
                        lg = hpool.tile([128, 40], dt.float32, tag="lg")
                        nc.vector.tensor_tensor(
                            out=lg[:], in0=ps40[:], in1=lbt[:], op=ALU.add)
                        m4 = spool.tile([128, 1], dt.float32, tag="m4")
                        nc.vector.reduce_max(
                            out=m4[:], in_=lg[:], axis=mybir.AxisListType.X)
                        negm4 = spool.tile([128, 1], dt.float32, tag="negm4")
                        nc.vector.tensor_scalar_mul(negm4[:], m4[:], -1.0)
                        e4 = hpool.tile([128, 40], dt.float32, tag="e4")
                        z4 = spool.tile([128, 1], dt.float32, tag="z4")
                        nc.scalar.activation(
                            out=e4[:], in_=lg[:], func=AF.Exp,
                            bias=negm4[:, 0:1], scale=1.0, accum_out=z4[:, 0:1],
                        )
                        lnz = spool.tile([128, 1], dt.float32, tag="lnz")
                        nc.scalar.activation(
                            out=lnz[:], in_=z4[:], func=AF.Ln, bias=0.0)
                        lgo = hpool.tile([128, 40], dt.float32, tag="lgo")
                        nc.vector.tensor_scalar(
                            out=lgo[:], in0=lg[:], scalar1=negm4[:, 0:1],
                            scalar2=lnz[:, 0:1], op0=ALU.add, op1=ALU.subtract,
                        )
                        nc.sync.dma_start(
                            out=logits[t * P:(t + 1) * P, :], in_=lgo[:])

            run_edge0()
            run_edge1()

    nc.finalize()
    return nc


def build_all(inputs):
    x = np.ascontiguousarray(np.asarray(inputs["x"], dtype=np.float32))
    edge = np.asarray(inputs["edge"])
    W0 = np.asarray(inputs["W0"], dtype=np.float32)
    a0 = np.asarray(inputs["a0"], dtype=np.float32).reshape(-1)
    W1 = np.asarray(inputs["W1"], dtype=np.float32)
    a1 = np.asarray(inputs["a1"], dtype=np.float32).reshape(-1)
    lin_w = np.asarray(inputs["lin_w"], dtype=np.float32)
    lin_b = np.asarray(inputs["lin_b"], dtype=np.float32)

    pre = _preprocess(edge)

    M0, M0inv = _householder_M(a0[256:512])
    M1, M1inv = _householder_M(a1[128:256])
    w0e = np.concatenate(
        [(W0.astype(np.float64) @ M0.astype(np.float64)).astype(np.float32),
         (W0 @ a0[:256]).reshape(-1, 1)], axis=1)          # [256, 257]
    w1e = np.concatenate(
        [(W1.astype(np.float64) @ M1.astype(np.float64)).astype(np.float32),
         (W1 @ a1[:128]).reshape(-1, 1)], axis=1)          # [256, 129]
    lb_rep = np.tile(lin_b[None, :], (128, 1)).astype(np.float32)
    pad0 = np.zeros((1, 256), np.float32); pad0[0, 255] = NEG
    pad1 = np.zeros((1, 128), np.float32); pad1[0, 127] = NEG

    in_maps = []
    for c in range(NC):
        xTc = np.zeros((256, POS), np.float32)
        xTc[:, :SH] = x[pre["perm"][c]].T
        in_maps.append({
            "xT": xTc,
            "eidx0": pre["eidx0"][c],
            "eidx1": pre["eidx1"][c],
            "scidx": np.ascontiguousarray(pre["scidx"][c]),
            "w0e": w0e, "w1e": w1e,
            "m0i": M0inv, "m1i": M1inv,
            "lw": lin_w, "lb": lb_rep,
            "pad0": pad0.astype(BF16), "pad1": pad1.astype(BF16),
        })

    nc = _build(pre["meta"])
    return nc, in_maps, pre


def _assemble(results, pre):
    out = np.empty((N, 40), np.float32)
    for c in range(NC):
        out[pre["perm"][c]] = results[c]["logits"][:SH]
    return out


def _ensure_device(max_tries=8, sleep_s=10.0):
    import time
    import jax

    for i in range(max_tries):
        try:
            a = jax.device_put(np.ones(8, np.float32))
            jax.block_until_ready(a + 1)
            return
        except Exception:  # noqa: BLE001
            if i == max_tries - 1:
                raise
            time.sleep(sleep_s)


def kernel(**inputs) -> np.ndarray:
    import time
    from concourse.bass_utils import run_bass_kernel_spmd

    nc, in_maps, pre = build_all(inputs)
    _ensure_device()
    last = None
    for _ in range(3):
        try:
            res = run_bass_kernel_spmd(nc, in_maps, list(range(NC)))
            return _assemble(res.results, pre)
        except Exception as e:  # noqa: BLE001
            last = e
            time.sleep(15.0)
            _ensure_device()
    raise last
